# revision 1
# baseline (speedup 1.0000x reference)
"""Multi-head attention (B=2, S=2048, D=1024, H=16) on 8 trn2 NeuronCores.

Tensor-parallel over heads (2 heads per core, column-sliced wq/wk/wv) for the
QKV projections and attention; a per-(batch, head-group) AllToAll then
redistributes the attention output so each core computes the output
projection for its own interleaved 512-row slice of the flattened (B*S)
sequence (Megatron-style TP with a sequence-parallel output projection).

Layout/engine choices:
  - the host supplies x.T and w.T so every matmul operand arrives K-major;
    no activation transposes on device
  - logits are computed transposed [t, s] so the softmax exp (over t) feeds
    the P@V matmul directly -- no probability-matrix transposes
  - ones-columns appended to V produce the softmax denominators in the same
    PV matmul (PSUM rows 64..127), replicated across partitions for a cheap
    vector normalize
  - matmuls run in float32r (full-rate relaxed fp32); the x/w stream and the
    projection tail (attnT, collective buffers, wo) are float16
  - exp runs on ACT from 2x[128,1024] double-buffered PSUM logit tiles --
    ACT is the attention-phase bottleneck, PE fills gaps with PV/logit mms
  - attention processes s in two half-passes so it needs only 6 PSUM banks;
    the freed 2 banks let batch-1's QKV projections and V-transposes fold
    into batch-0's ACT-bound attention window (PE and ACT both ~95% busy)
  - the four 0.25MB AllToAlls overlap attention; both output projections
    run in the tail, overlapping the only exposed (last) collective
"""

import sys

sys.path.insert(0, "/opt/trn_rl_repo")

import numpy as np

import concourse.mybir as mybir
import concourse.tile as tile
from concourse import bacc
from concourse.bass_utils import run_bass_kernel_spmd
from concourse.masks import make_identity

B, S, D = 2, 2048, 1024
H, HD = 16, 64
NCORES = 8
DL = D // NCORES          # 128 local attn dims (2 heads) per core
R = B * S                 # 4096 flattened rows
RSL = R // NCORES         # 512 output rows per core
P = 128
KC = D // P               # 8 contraction chunks of 128
TC = S // P               # 16 key/t chunks per batch
SB = 512                  # moving-operand (N) tile
NSB = S // SB             # 4 s-chunks per batch
F32 = mybir.dt.float32
F32R = mybir.dt.float32r
F16 = mybir.dt.float16

_CACHE = {}


def _build(n_iters=1, phases=3, bench=False):
    nc = bacc.Bacc("TRN2", target_bir_lowering=False, debug=False,
                   num_devices=NCORES)
    Exp = mybir.ActivationFunctionType.Exp

    kind = "Internal" if bench else "ExternalInput"
    xT = nc.dram_tensor("xT", [D, R], F16, kind=kind)
    wqT = nc.dram_tensor("wqT", [D, DL], F16, kind=kind)
    wkT = nc.dram_tensor("wkT", [D, DL], F16, kind=kind)
    wvT = nc.dram_tensor("wvT", [D, DL], F16, kind=kind)
    woT = nc.dram_tensor("woT", [D, D], F16, kind=kind)
    bqkv = nc.dram_tensor("bqkv", [DL, 3], F32, kind=kind)
    bo_t = nc.dram_tensor("bo_t", [P, NCORES], F32, kind=kind)
    out = nc.dram_tensor("out", [D, RSL], F32, kind="ExternalOutput")

    with tile.TileContext(nc) as tc:
        with (
            tc.tile_pool(name="const", bufs=1) as const,
            tc.tile_pool(name="persist", bufs=1) as persist,
            tc.tile_pool(name="dram", bufs=1, space="DRAM") as dram,
        ):
            # ---- constants / weights resident in SBUF ----
            ident = const.tile([P, P], F16, tag="ident")
            make_identity(nc, ident[:])
            bias3 = const.tile([DL, 3], F32, tag="bias3")
            bo_s = const.tile([P, NCORES], F32, tag="bo_s")
            if bench:
                nc.vector.memset(bias3[:], 0.0)
                nc.vector.memset(bo_s[:], 0.0)
            else:
                nc.sync.dma_start(bias3[:], bqkv[:])
                nc.sync.dma_start(bo_s[:], bo_t[:])

            w_s = []
            for name in ("wq", "wk", "wv"):
                w_s.append(const.tile([P, D], F16, tag=f"w_{name}",
                                      name=f"w_{name}"))
            for t, wt in ((w_s[0], wqT),):
                if bench:
                    nc.vector.memset(t[:], 0.0)
                else:
                    nc.sync.dma_start(
                        t[:].rearrange("p (kc c) -> p kc c", c=P),
                        wt.rearrange("(kc p) c -> p kc c", p=P))
            wo_s = [const.tile([P, D], F16, tag=f"wo{kc}", name=f"wo{kc}")
                    for kc in range(KC)]

            # persistent activations
            QT = persist.tile([P, R], F32R, tag="QT")   # [2 heads*64, B*S]
            KT = persist.tile([P, R], F32R, tag="KT")
            VT = persist.tile([P, R], F16, tag="VT")
            # V natural per 128-row t-chunk: [v_h0 |ones| v_h1 |ones]
            vn = persist.tile([P, (R // P) * 256], F16, tag="vn")
            vn3 = vn[:].rearrange("p (g two c) -> p g two c", two=2, c=128)
            nc.vector.memset(vn3[:, :, :, 64:128], 1.0)
            attnT = persist.tile([P, R], F16, tag="attnT")

            for it in range(n_iters):
                SH = S // 2
                CW = RSL // 2
                a2a_in = [[dram.tile([NCORES, HD, CW], F16,
                                     tag=f"a2a_in{it}_{b}_{h}",
                                     name=f"a2a_in{it}_{b}_{h}")
                           for h in range(2)] for b in range(B)]
                a2a_out = [[dram.tile([NCORES, HD, CW], F16,
                                      tag=f"a2a_out{it}_{b}_{h}",
                                      name=f"a2a_out{it}_{b}_{h}")
                            for h in range(2)] for b in range(B)]

                def load_half(half, xt_pool):
                    hof = half * (R // 2)
                    xts = []
                    for kc in range(KC):
                        t = xt_pool.tile([P, R // 2], F16, tag="xt",
                                         name=f"xt_{it}_{half}_{kc}")
                        nc.sync.dma_start(
                            t[:], xT[kc * P:(kc + 1) * P, hof:hof + R // 2])
                        xts.append(t)
                        if it == 0 and half == 0 and kc == 0:
                            # wk/wv ride the queue behind the first x tile:
                            # the first q-matmuls only need wq + x[0]
                            for wtile, wt in ((w_s[1], wkT), (w_s[2], wvT)):
                                if bench:
                                    nc.vector.memset(wtile[:], 0.0)
                                else:
                                    nc.sync.dma_start(
                                        wtile[:].rearrange(
                                            "p (kc c) -> p kc c", c=P),
                                        wt.rearrange("(kc p) c -> p kc c",
                                                     p=P))
                    return xts

                def qkv_copy(pj, i, nb, hof, ps):
                    dst_ap = (QT, KT, VT)[pj][:, hof + nb * SB:
                                              hof + (nb + 1) * SB]
                    if (pj + i) % 2 == 0:
                        nc.vector.tensor_scalar_add(dst_ap, ps[:],
                                                    bias3[:, pj:pj + 1])
                    else:
                        nc.scalar.add(dst_ap, ps[:], bias3[:, pj:pj + 1])

                def vnat(half, pool, tag):
                    # V natural (+ ones) tiles for this half's t-chunks
                    for g in range(half * 16, half * 16 + 16):
                        pt = pool.tile([P, P], F16, tag=tag,
                                       name=f"pt_{it}_{half}_{g}")
                        nc.tensor.transpose(pt[:], VT[:, g * P:(g + 1) * P],
                                            ident[:])
                        o = g * 256
                        nc.vector.tensor_copy(vn[:, o:o + 64], pt[:, 0:64])
                        nc.vector.tensor_copy(vn[:, o + 128:o + 192],
                                              pt[:, 64:128])

                def attention_batch(b, ps3, exps, norm):
                    base = b * S
                    for h in range(2):
                        hr = slice(h * HD, (h + 1) * HD)
                        for sh in range(2):
                            sof = base + sh * SH
                            pv = ps3.tile([P, SH], F32, tag="pv", bufs=1,
                                          name=f"pv_{it}_{b}_{h}_{sh}")
                            for tcn in range(TC):
                                ex = exps.tile([P, SH], F16, tag="ex",
                                               name=f"ex_{it}_{b}_{h}_{sh}_{tcn}")
                                lg = ps3.tile([P, SH], F32, tag="lg", bufs=2,
                                              name=f"lg_{it}_{b}_{h}_{sh}_{tcn}")
                                for sb in range(2):
                                    nc.tensor.matmul(
                                        lg[:, sb * SB:(sb + 1) * SB],
                                        KT[hr, base + tcn * P:
                                           base + (tcn + 1) * P],
                                        QT[hr, sof + sb * SB:
                                           sof + (sb + 1) * SB],
                                        start=True, stop=True)
                                nc.scalar.activation(ex[:], lg[:], Exp,
                                                     scale=1.0 / 8.0)
                                o = (b * TC + tcn) * 256 + h * 128
                                for sb in range(2):
                                    nc.tensor.matmul(
                                        pv[:, sb * SB:(sb + 1) * SB],
                                        vn[:, o:o + 128],
                                        ex[:, sb * SB:(sb + 1) * SB],
                                        start=(tcn == 0), stop=(tcn == TC - 1))
                            vcp = norm.tile([P, SH], F32, tag="vcp")
                            nc.vector.tensor_copy(vcp[:], pv[:])
                            rc = norm.tile([HD, SH], F32, tag="rc")
                            nc.vector.reciprocal(rc[:], vcp[64:128, :])
                            nc.vector.tensor_mul(
                                attnT[h * HD:(h + 1) * HD, sof:sof + SH],
                                vcp[0:64, :], rc[:])
                        # ship this (batch, head) chunk; overlaps compute
                        if phases >= 3:
                            nc.sync.dma_start(
                                a2a_in[b][h].rearrange("j p c -> p j c"),
                                attnT[h * HD:(h + 1) * HD,
                                      base:base + S].rearrange(
                                          "p (j c) -> p j c", c=CW))
                            nc.gpsimd.collective_compute(
                                "AllToAll", mybir.AluOpType.bypass,
                                replica_groups=[list(range(NCORES))],
                                ins=[a2a_in[b][h].opt()],
                                outs=[a2a_out[b][h].opt()])

                def proj_batch(b, proj, ps4, outs):
                    rh_b = proj.tile([P, KC * CW], F16, tag=f"rh{it}_{b}",
                                     name=f"rh{it}_{b}")
                    for h in range(2):
                        nc.sync.dma_start(
                            rh_b[h * HD:(h + 1) * HD, :].rearrange(
                                "p (kc c) -> p kc c", c=CW),
                            a2a_out[b][h].rearrange("kc p c -> p kc c"))
                    for mc in range(KC):
                        ps = ps4.tile([P, CW], F32, tag="ps4",
                                      name=f"ps4_{it}_{b}_{mc}")
                        for kc in range(KC):
                            nc.tensor.matmul(
                                ps[:], wo_s[kc][:, mc * P:(mc + 1) * P],
                                rh_b[:, kc * CW:(kc + 1) * CW],
                                start=(kc == 0), stop=(kc == KC - 1))
                        ot = outs.tile([P, CW], F32, tag="ot",
                                       name=f"ot_{it}_{b}_{mc}")
                        nc.vector.tensor_scalar_add(ot[:], ps[:],
                                                    bo_s[:, mc:mc + 1])
                        nc.sync.dma_start(
                            out[mc * P:(mc + 1) * P, b * CW:(b + 1) * CW],
                            ot[:])

                with tc.tile_pool(name=f"xt{it}", bufs=8) as xt_pool:
                    # ---- batch-0 QKV + V-transposes (full-width PSUM) ----
                    with (
                        tc.tile_pool(name=f"ps1{it}", bufs=6,
                                     space="PSUM") as ps1,
                        tc.tile_pool(name=f"pst{it}", bufs=2,
                                     space="PSUM") as pst,
                    ):
                        xts0 = load_half(0, xt_pool)
                        for np_ in range(2):
                            pss = [[ps1.tile([P, SB], F32, tag="ps1",
                                             name=f"ps1_{it}_0_{np_}_{pj}_{i}")
                                    for i in range(2)] for pj in range(3)]
                            for kc in range(KC):
                                for pj in range(3):
                                    for i in range(2):
                                        nb = np_ * 2 + i
                                        nc.tensor.matmul(
                                            pss[pj][i][:],
                                            w_s[pj][:, kc * P:(kc + 1) * P],
                                            xts0[kc][:, nb * SB:(nb + 1) * SB],
                                            start=(kc == 0),
                                            stop=(kc == KC - 1))
                            for pj in range(3):
                                for i in range(2):
                                    qkv_copy(pj, i, np_ * 2 + i, 0,
                                             pss[pj][i])
                        vnat(0, pst, "pst")

                    for kc in range(KC):
                        if bench:
                            nc.vector.memset(wo_s[kc][:], 0.0)
                        else:
                            nc.sync.dma_start(
                                wo_s[kc][:], woT[kc * P:(kc + 1) * P, :])
                    if phases < 2:
                        continue

                    with (
                        tc.tile_pool(name=f"ps3{it}", bufs=1,
                                     space="PSUM") as ps3,
                        tc.tile_pool(name=f"exps{it}", bufs=4) as exps,
                        tc.tile_pool(name=f"norm{it}", bufs=2) as norm,
                    ):
                        # attention b0 (6 banks); QKV-half1 gap-fills PE
                        attention_batch(0, ps3, exps, norm)

                        with tc.tile_pool(name=f"ps1b{it}", bufs=2,
                                          space="PSUM") as ps1b:
                            xts1 = load_half(1, xt_pool)
                            hof = R // 2
                            for pj in range(3):
                                for nb in range(4):
                                    t = ps1b.tile([P, SB], F32, tag="ps1b",
                                                  name=f"ps1b_{it}_{pj}_{nb}")
                                    for kc in range(KC):
                                        nc.tensor.matmul(
                                            t[:],
                                            w_s[pj][:, kc * P:(kc + 1) * P],
                                            xts1[kc][:, nb * SB:(nb + 1) * SB],
                                            start=(kc == 0),
                                            stop=(kc == KC - 1))
                                    qkv_copy(pj, nb % 2, nb, hof, t)
                            vnat(1, ps1b, "ps1b")

                        attention_batch(1, ps3, exps, norm)

                if phases < 3:
                    continue
                with (
                    tc.tile_pool(name=f"proj1{it}", bufs=1) as proj1,
                    tc.tile_pool(name=f"ps41{it}", bufs=4,
                                 space="PSUM") as ps41,
                    tc.tile_pool(name=f"outs1{it}", bufs=4) as outs1,
                ):
                    proj_batch(0, proj1, ps41, outs1)
                    proj_batch(1, proj1, ps41, outs1)

    nc.compile()
    return nc


def _get_program(n_iters=1, phases=3, bench=False):
    key = (n_iters, phases, bench)
    if key not in _CACHE:
        _CACHE[key] = _build(n_iters, phases, bench)
    return _CACHE[key]


def _in_maps(x, wq, bq, wk, bk, wv, bv, wo, bo):
    x = np.asarray(x, np.float32)
    xT = np.ascontiguousarray(x.reshape(R, D).T.astype(np.float16))
    woT = np.ascontiguousarray(
        np.asarray(wo, np.float32).T.astype(np.float16))
    bo_t = np.ascontiguousarray(
        np.asarray(bo, np.float32).reshape(NCORES, P).T)
    maps = []
    for i in range(NCORES):
        sl = slice(i * DL, (i + 1) * DL)
        maps.append({
            "xT": xT,
            "wqT": np.ascontiguousarray(np.asarray(wq, np.float32)[sl, :].T
                                        .astype(np.float16)),
            "wkT": np.ascontiguousarray(np.asarray(wk, np.float32)[sl, :].T
                                        .astype(np.float16)),
            "wvT": np.ascontiguousarray(np.asarray(wv, np.float32)[sl, :].T
                                        .astype(np.float16)),
            "woT": woT,
            "bqkv": np.ascontiguousarray(np.stack(
                [np.asarray(bq, np.float32)[sl],
                 np.asarray(bk, np.float32)[sl],
                 np.asarray(bv, np.float32)[sl]], axis=1)),
            "bo_t": bo_t,
        })
    return maps


def kernel(x, wq, bq, wk, bk, wv, bv, wo, bo, **_):
    nc = _get_program()
    res = run_bass_kernel_spmd(nc, _in_maps(x, wq, bq, wk, bk, wv, bv, wo, bo),
                               list(range(NCORES)))
    # core j holds, for each batch b, output columns
    # [b*2048 + j*256, b*2048 + (j+1)*256) of out.T
    CW = RSL // 2
    outT = np.empty((D, R), np.float32)
    for j in range(NCORES):
        o = res.results[j]["out"]
        for b in range(B):
            outT[:, b * S + j * CW:(b * S) + (j + 1) * CW] = \
                o[:, b * CW:(b + 1) * CW]
    return np.ascontiguousarray(outT.T).reshape(B, S, D)

